# revision 48
# baseline (speedup 1.0000x reference)
"""CLVP self-attention Trainium2 kernel (8 NeuronCores, SPMD).

Sharding: batch x head-group. Core c handles batch b = c//2 and heads
hg*8..hg*8+7 where hg = c%2. Each core computes its 8 heads' attention for
its batch plus the partial output projection; the host sums the two
head-group partials per batch and adds the output bias.

Device-side layout strategy:
  - All matmul contractions put the contracted dim on SBUF partitions.
  - Q^T/K^T are produced in [channel, seq] layout directly (channel on
    partitions) so QK^T needs no transposes; scores come out as
    S^T = [s_k partitions, s_q free], so softmax's sum reduction is done
    by an extra all-ones channel appended to V in the P^T @ V matmul.
  - V is produced in natural [seq, channel] layout (+ ones column) and is
    the stationary operand of the PV matmul.
  - RoPE (q, k, and v all get it in this module) is applied with
    elementwise multiply-adds against host-precomputed cos/sin tiles.
  - Causal masking: above-diagonal 128-col sub-blocks are skipped
    entirely (QK matmul + exp restricted to the needed query range);
    diagonal blocks are masked after exp via gpsimd affine_select, which
    also zero-fills the skipped (stale) regions.
  - exp() needs no max-subtraction: scores are ~N(0, 0.41^2) for this
    problem's distributions, so exp is numerically safe; the attention
    scale (D^-0.5) is folded into the host-side q rope tables.
  - Softmax normalization: the PV matmul's ones-channel yields rowsums in
    a PSUM row; a scalar-engine Reciprocal turns them into 1/den, and a
    tiny block-diagonal ones matmul broadcasts them across the 128 ctx
    partitions (PE does the broadcast, not gpsimd), so the normalize is
    two fused vector multiplies per head-pair.
  - The attention inner loop is software-pipelined (QK of block jp+1 is
    issued before PV of block jp) so the tensor engine never idles
    waiting for the scalar-engine exp; chunk qc=1's attention overlaps
    chunk qc=0's normalization, and both output projections run at the
    end back to back.
  - Matmul operands are float16 (full PE rate, fp32 PSUM accumulation).
"""

import os
import sys

import numpy as np

for _p in (
    "/root/.axon_site",
    "/root/.axon_site/_ro/trn_rl_repo",
    "/root/.axon_site/_ro/pypackages",
    "/opt/trn_rl_repo",
):
    if os.path.isdir(_p) and _p not in sys.path:
        sys.path.append(_p)

import concourse.bass as bass  # noqa: E402
import concourse.tile as tile  # noqa: E402
from concourse import bacc, mybir  # noqa: E402

B, S, E, H = 4, 1024, 1024, 16
D = E // H          # 64 head dim
ROT = 32            # rotary channels per head
HALF = ROT // 2     # 16
NCORES = 8
HPC = H // 2        # 8 heads per core
CPC = HPC * D       # 512 channels per core
NT = CPC // 128     # 4 channel tiles (2 heads each)
ST = S // 128       # 8 seq tiles
ET = E // 128       # 8 embed (contraction) tiles
VW = D + 1          # 65: v channels + ones column

F32 = mybir.dt.float32
F16 = mybir.dt.float16


def build_nc():
    # Bacc (not raw Bass): its compile() pass moves extra matmul waits onto
    # LdWeights — walrus allows only one sync wait per Matmult instruction.
    nc = bacc.Bacc("TRN2", target_bir_lowering=False)
    xt_d = nc.dram_tensor("xt", [E, S], F16, kind="ExternalInput")
    wq_d = nc.dram_tensor("wqT", [E, CPC], F16, kind="ExternalInput")
    wk_d = nc.dram_tensor("wkT", [E, CPC], F16, kind="ExternalInput")
    wv_d = nc.dram_tensor("wvT", [E, CPC], F16, kind="ExternalInput")
    wo_d = nc.dram_tensor("woT", [CPC, E], F16, kind="ExternalInput")
    cos_d = nc.dram_tensor("cos_t", [D, S], F16, kind="ExternalInput")
    sin_d = nc.dram_tensor("sin_t", [D, S], F16, kind="ExternalInput")
    cosv_d = nc.dram_tensor("cosv", [128, ST, ROT], F16, kind="ExternalInput")
    sinv_d = nc.dram_tensor("sinv", [128, ST, ROT], F16, kind="ExternalInput")
    out_d = nc.dram_tensor("out", [S, E], F16, kind="ExternalOutput")

    from contextlib import ExitStack

    with tile.TileContext(nc) as tc, ExitStack() as ctx:
        consts = ctx.enter_context(tc.tile_pool(name="consts", bufs=1))
        wpool = ctx.enter_context(tc.tile_pool(name="wpool", bufs=24))
        # pt tiles persist across the decoupled QK/PV phases: up to 8 per
        # head pair (qc=1), three head pairs in flight
        ptpool = ctx.enter_context(tc.tile_pool(name="ptpool", bufs=26))
        opool = ctx.enter_context(tc.tile_pool(name="opool", bufs=3))
        vspool = ctx.enter_context(tc.tile_pool(name="vspool", bufs=2))
        rcppool = ctx.enter_context(tc.tile_pool(name="rcppool", bufs=4))
        # PSUM budget (8 banks of [128,512]f32):
        #   psq  4 bufs — attention qk score tiles (j and j+1, both heads)
        #   pspj 2 bufs — projection chunks / out-proj / rcp broadcast
        #   pspv 2 bufs — PV accumulators (pva, pvb)
        psq = ctx.enter_context(tc.tile_pool(name="psq", bufs=2, space="PSUM"))
        pspj = ctx.enter_context(tc.tile_pool(name="pspj", bufs=2, space="PSUM"))
        pspv = ctx.enter_context(tc.tile_pool(name="pspv", bufs=2, space="PSUM"))

        # ---- persistent SBUF tensors -------------------------------------
        xt_sb = consts.tile([128, ET, S], F16, tag="xt")
        wo_sb = consts.tile([128, NT, E], F16, tag="wo")
        cos_sb = consts.tile([128, S], F16, tag="cos")
        sin_sb = consts.tile([128, S], F16, tag="sin")
        cosv_sb = consts.tile([128, ST, ROT], F16, tag="cosv")
        sinv_sb = consts.tile([128, ST, ROT], F16, tag="sinv")

        qt_sb = consts.tile([128, NT, S], F16, tag="qt")
        kt_sb = consts.tile([128, NT, S], F16, tag="kt")
        vp_sb = consts.tile([128, ST, HPC, VW], F16, tag="vp")
        ctx_sb = consts.tile([128, NT, S], F16, tag="ctx")



        # ones column of V (denominator channel)
        nc.vector.memset(vp_sb[:, :, :, D : D + 1], 1.0)

        # lower-triangular 0/1 mask for the diagonal attention blocks (the
        # only gpsimd op in the kernel — keeping gpsimd to a single ucode
        # avoids its very expensive per-switch library reloads)
        tril = consts.tile([128, 128], F16, tag="tril")
        nc.vector.memset(tril[:], 1.0)
        nc.gpsimd.affine_select(
            out=tril[:],
            in_=tril[:],
            pattern=[[1, 128]],
            compare_op=mybir.AluOpType.is_ge,
            fill=0.0,
            base=0,
            channel_multiplier=-1,
        )

        # rope half-swap as a 32-lane partition shuffle: output lane i of
        # each 32-partition group reads lane (i+16)%32.  Non-rotary lanes
        # get shuffled garbage, but their sin table entries are 0.
        SHUF = [(i + HALF) % 32 for i in range(32)]
        xspool = ctx.enter_context(tc.tile_pool(name="xspool", bufs=2))

        # preload the gpsimd partition_broadcast ucode during the DMA fill
        # so the first real broadcast doesn't pay the library load
        pre_in = consts.tile([1, 8], F16, tag="pre_in")
        pre_out = consts.tile([128, 8], F16, tag="pre_out")
        nc.vector.memset(pre_in[:], 0.0)
        nc.gpsimd.partition_broadcast(pre_out[:], pre_in[:], channels=128)

        # ---- input DMAs, in consumption order, split over two queues -----
        # sync queue: xt/wv interleaved (V projection consumes them first),
        # then wo.  scalar queue: wq, rope tables, wk — in parallel with
        # the sync stream.  Rope-shift DMAs later go on the gpsimd queue so
        # they never wait behind input loads.
        wv_t, wq_t, wk_t = [], [], []
        for e in range(ET):
            nc.sync.dma_start(
                out=xt_sb[:, e, :], in_=xt_d[e * 128 : (e + 1) * 128, :]
            )
            w = wpool.tile([128, CPC], F16, tag="w")
            nc.sync.dma_start(out=w[:], in_=wv_d[e * 128 : (e + 1) * 128, :])
            wv_t.append(w)
        for e in range(ET):
            w = wpool.tile([128, CPC], F16, tag="w")
            nc.scalar.dma_start(out=w[:], in_=wq_d[e * 128 : (e + 1) * 128, :])
            wq_t.append(w)
        for e in range(ET):
            w = wpool.tile([128, CPC], F16, tag="w")
            nc.scalar.dma_start(out=w[:], in_=wk_d[e * 128 : (e + 1) * 128, :])
            wk_t.append(w)
        # rope tables: rows repeat with period 64, so ship [64, S] once and
        # land it in both partition halves
        nc.scalar.dma_start(out=cos_sb[0:D, :], in_=cos_d[:])
        nc.scalar.dma_start(out=cos_sb[D:128, :], in_=cos_d[:])
        nc.scalar.dma_start(out=sin_sb[0:D, :], in_=sin_d[:])
        nc.scalar.dma_start(out=sin_sb[D:128, :], in_=sin_d[:])
        nc.scalar.dma_start(out=cosv_sb[:], in_=cosv_d[:])
        nc.scalar.dma_start(out=sinv_sb[:], in_=sinv_d[:])
        for t in range(NT):
            nc.sync.dma_start(
                out=wo_sb[:, t, :], in_=wo_d[t * 128 : (t + 1) * 128, :]
            )

        # ---- V projection (natural [s, c] layout) ------------------------
        def gen_vproj():
            for st in range(ST):
                pv = pspj.tile([128, 512], F32, tag="pj")
                for e in range(ET):
                    nc.tensor.matmul(
                        pv[:],
                        (xt_sb[:, e, st * 128 : (st + 1) * 128]),
                        (wv_t[e][:]),
                        start=(e == 0),
                        stop=(e == ET - 1),
                    )
                    if e == 3:
                        yield
                # strided copy into vp (64 of each head's 65 columns)
                nc.vector.tensor_copy(
                    vp_sb[:, st, :, 0:D],
                    pv[:].rearrange("p (h c) -> p h c", h=HPC),
                )
                # rope shifted operand (swap the two 16-halves of the rot
                # channels)
                vs = vspool.tile([128, HPC, ROT], F16, tag="vs")
                pvh = pv[:].rearrange("p (h c) -> p h c", h=HPC)
                nc.vector.tensor_copy(vs[:, :, 0:HALF], pvh[:, :, HALF:ROT])
                nc.vector.tensor_copy(vs[:, :, HALF:ROT], pvh[:, :, 0:HALF])
                # v = v*cos + vs*sin   (cos/sin broadcast across heads)
                cosb = cosv_sb[:, st, None, :].to_broadcast((128, HPC, ROT))
                sinb = sinv_sb[:, st, None, :].to_broadcast((128, HPC, ROT))
                nc.vector.tensor_tensor(
                    vp_sb[:, st, :, 0:ROT],
                    vp_sb[:, st, :, 0:ROT],
                    cosb,
                    mybir.AluOpType.mult,
                )
                nc.vector.tensor_tensor(
                    vs[:], vs[:], sinb, mybir.AluOpType.mult
                )
                nc.vector.tensor_tensor(
                    vp_sb[:, st, :, 0:ROT],
                    vp_sb[:, st, :, 0:ROT],
                    vs[:],
                    mybir.AluOpType.add,
                )
                yield

        # ---- interleaved phase machinery ---------------------------------
        # Each generator issues one dependency-chunk of instructions per
        # next(); drive() round-robins them so the tensor-engine stream
        # always has independent filler work between dependent groups
        # (keeps the PE dense -> HAM stays un-throttled).
        def drive(*gens):
            live = [iter(g) for g in gens]
            while live:
                for g in list(live):
                    try:
                        next(g)
                    except StopIteration:
                        live.remove(g)

        def gen_proj(ct):
            """Q^T / K^T projection for channel tile ct ([c, s] layout).
            The attention scale rides the q psum->sbuf copy for free."""
            for dst_sb, w_t, scale in (
                (qt_sb, wq_t, float(D) ** -0.5),
                (kt_sb, wk_t, 1.0),
            ):
                for sc in range(2):
                    scs = slice(sc * 512, sc * 512 + 512)
                    pq = pspj.tile([128, 512], F32, tag="pj")
                    for e in range(ET):
                        nc.tensor.matmul(
                            pq[:],
                            (w_t[e][:, ct * 128 : (ct + 1) * 128]),
                            (xt_sb[:, e, scs]),
                            start=(e == 0),
                            stop=(e == ET - 1),
                        )
                        if e == 3:
                            yield
                    nc.scalar.activation(
                        dst_sb[:, ct, scs],
                        pq[:],
                        mybir.ActivationFunctionType.Copy,
                        scale=scale,
                    )
                    yield
                # rope shifted operand via one partition shuffle, then
                # q *= cos, xs *= sin, q += xs
                xs = xspool.tile([128, S], F16, tag="xs")
                nc.vector.stream_shuffle(xs[:], dst_sb[:, ct, :], SHUF)
                nc.vector.tensor_tensor(
                    dst_sb[:, ct, :],
                    dst_sb[:, ct, :],
                    cos_sb[:],
                    mybir.AluOpType.mult,
                )
                nc.vector.tensor_tensor(
                    xs[:], xs[:], sin_sb[:], mybir.AluOpType.mult
                )
                nc.vector.tensor_tensor(
                    dst_sb[:, ct, :],
                    dst_sb[:, ct, :],
                    xs[:],
                    mybir.AluOpType.add,
                )
                yield

        def issue_qk(qc, t, j):
            """QK matmuls + exp + diagonal mask for k-tile j.  Both heads
            of the pair go into one [128, 1024] tile (head hh at columns
            hh*512..); the two matmuls are row-tiled (array rows 0-63 /
            64-127) so they run concurrently in the PE array.  Returns the
            pt tile and the low query bound."""
            lo = max(0, j - 4 * qc) * 128  # skip above-diagonal blocks
            qk = psq.tile([128, 1024], F32, tag="qk")
            for hh, base in ((0, 0), (1, 64)):
                hsl = slice(base, base + D)
                nc.tensor.matmul(
                    qk[:, hh * 512 + lo : hh * 512 + 512],
                    (kt_sb[hsl, t, j * 128 : (j + 1) * 128]),
                    (qt_sb[hsl, t, qc * 512 + lo : qc * 512 + 512]),
                    start=True,
                    stop=True,
                    tile_position=(base, 0),
                )
            pt = ptpool.tile([128, 1024], F16, tag="pt")
            w = 512 - lo
            # one strided exp over both heads' live columns
            nc.scalar.activation(
                pt[:].rearrange("p (h q) -> p h q", h=2)[:, :, lo:512],
                qk[:].rearrange("p (h q) -> p h q", h=2)[:, :, lo:512],
                mybir.ActivationFunctionType.Exp,
            )
            dg = j - 4 * qc  # diagonal query-subtile index (if 0..3)
            if 0 <= dg <= 3:  # tril-mask the diagonal 128x128 blocks
                ptd = pt[:].rearrange("p (h q) -> p h q", h=2)[
                    :, :, dg * 128 : dg * 128 + 128
                ]
                trb = tril[:, None, :].to_broadcast((128, 2, 128))
                nc.vector.tensor_tensor(ptd, ptd, trb, mybir.AluOpType.mult)
            return pt, lo

        pts_store = {}

        def gen_qkphase(qc, t):
            """All QK matmuls + exps for (qc, t), into persistent pt tiles.
            Decoupled from the PV phase so PV never waits on a fresh exp."""
            nj = 4 * qc + 4  # causal: k-tiles 0..4qc+3
            pts = []
            for j in range(nj):
                pts.append(issue_qk(qc, t, j))
                yield
            pts_store[(qc, t)] = pts

        def gen_pvphase(qc, t):
            """All PV matmuls for (qc, t) (exps long since done), then the
            drain + normalization."""
            qs = slice(qc * 512, qc * 512 + 512)
            nj = 4 * qc + 4
            pts = pts_store.pop((qc, t))
            pva = pspv.tile([128, 512], F32, tag="pv", name="pva")
            pvb = pspv.tile([128, 512], F32, tag="pv", name="pvb")
            for j in range(nj):
                pt_cur, lo_cur = pts[j]
                for hh, pvx in ((0, pva), (1, pvb)):
                    nc.tensor.matmul(
                        pvx[0:VW, lo_cur:512],
                        (vp_sb[:, j, 2 * t + hh, :]),
                        (pt_cur[:, hh * 512 + lo_cur : hh * 512 + 512]),
                        start=(j == 0),
                        stop=(j == nj - 1),
                    )
                if j % 2 == 1:
                    yield
            # ---- normalization for this head pair ------------------------
            # Drain PSUM fast (scalar ctx copies + vector rowsum copies) so
            # the accumulators free up for the next head pair, then finish
            # the normalize on sbuf off the tensor-engine critical path:
            # reciprocal -> f16 -> gpsimd partition-broadcast -> 2 mults.
            nc.scalar.copy(ctx_sb[0:D, t, qs], pva[0:D, :])
            nc.scalar.copy(ctx_sb[D:128, t, qs], pvb[0:D, :])
            rs = rcppool.tile([1, 1024], F32, tag="rs")
            nc.vector.tensor_copy(rs[0:1, 0:512], pva[D : D + 1, :])
            nc.vector.tensor_copy(rs[0:1, 512:1024], pvb[D : D + 1, :])
            yield
            rc32 = rcppool.tile([1, 1024], F32, tag="rc32")
            nc.vector.reciprocal_approx_fast(rc32[:], rs[:])
            rcp = rcppool.tile([1, 1024], F16, tag="rcp")
            nc.vector.tensor_copy(rcp[:], rc32[:])
            bcast = rcppool.tile([128, 1024], F16, tag="bcast")
            # gpsimd partition-broadcast: the only gpsimd compute ucode in
            # the steady state, so no per-call library reload
            nc.gpsimd.partition_broadcast(bcast[:], rcp[:], channels=128)
            nc.vector.tensor_tensor(
                ctx_sb[0:D, t, qs],
                ctx_sb[0:D, t, qs],
                bcast[0:D, 0:512],
                mybir.AluOpType.mult,
            )
            nc.vector.tensor_tensor(
                ctx_sb[D:128, t, qs],
                ctx_sb[D:128, t, qs],
                bcast[D:128, 512:1024],
                mybir.AluOpType.mult,
            )
            yield

        def gen_outproj(groups):
            for gi, (ss, ec) in enumerate(groups):
                po = pspj.tile([128, 512], F32, tag="pj", name="po")
                for t2 in range(NT):
                    nc.tensor.matmul(
                        po[:],
                        (ctx_sb[:, t2, ss * 128 : (ss + 1) * 128]),
                        (wo_sb[:, t2, ec * 512 : ec * 512 + 512]),
                        start=(t2 == 0),
                        stop=(t2 == NT - 1),
                    )
                yield
                ot = opool.tile([128, 512], F16, tag="ot")
                # alternate the psum drain between scalar and vector so
                # neither engine becomes the tail bottleneck
                if gi % 2 == 0:
                    nc.scalar.copy(ot[:], po[:])
                else:
                    nc.vector.tensor_copy(ot[:], po[:])
                nc.sync.dma_start(
                    out=out_d[
                        ss * 128 : (ss + 1) * 128, ec * 512 : ec * 512 + 512
                    ],
                    in_=ot[:],
                )
                yield

        # Schedule: QK-phase(n) runs interleaved with PV-phase(n-1) plus
        # projection / output-projection filler, so every tensor-engine
        # burst is multi-us and nothing waits on a just-issued exp.
        og = [(ss, ec) for ss in range(ST) for ec in range(2)]
        drive(gen_vproj(), gen_proj(0))
        drive(gen_qkphase(0, 0), gen_proj(1))
        drive(gen_pvphase(0, 0), gen_qkphase(0, 1), gen_proj(2))
        drive(gen_pvphase(0, 1), gen_qkphase(0, 2), gen_proj(3))
        drive(gen_pvphase(0, 2), gen_qkphase(0, 3))
        drive(gen_pvphase(0, 3), gen_qkphase(1, 0))
        # output projection: qc=0 groups (og[0:8]) have their ctx ready a
        # full drive after norm(0,3) starts; qc=1 groups (og[8:16]) all
        # wait on norm(1,3), so the last qc=0 groups lead the final drive
        # to cover that latency.
        drive(gen_pvphase(1, 0), gen_qkphase(1, 1))
        drive(gen_pvphase(1, 1), gen_qkphase(1, 2), gen_outproj(og[0:2]))
        drive(gen_pvphase(1, 2), gen_qkphase(1, 3), gen_outproj(og[2:4]))
        drive(gen_pvphase(1, 3), gen_outproj(og[4:6]))
        drive(gen_outproj(og[6:16]))

    nc.compile()
    return nc


# ---------------------------------------------------------------------------
# host-side input prep


def _prep_consts(rotary_pos_emb):
    freqs = np.asarray(rotary_pos_emb, np.float32).reshape(S, ROT)
    cosf = np.cos(freqs)  # [S, ROT]
    sinf = np.sin(freqs)
    # channel-partition layout [64, S]: partition p holds channel p (the
    # device lands the same table in both partition halves)
    cos_ch = np.ones((D, S), np.float32)
    sin_ch = np.zeros((D, S), np.float32)
    for c in range(ROT):
        cos_ch[c] = cosf[:, c]
        sin_ch[c] = -sinf[:, c] if c < HALF else sinf[:, c]
    # natural layout for v rope: [128 (s within tile), ST, ROT]
    cosv = np.empty((128, ST, ROT), np.float32)
    sinv = np.empty((128, ST, ROT), np.float32)
    for st in range(ST):
        srows = slice(st * 128, st * 128 + 128)
        cosv[:, st, :] = cosf[srows]
        sinv[:, st, :HALF] = -sinf[srows, :HALF]
        sinv[:, st, HALF:] = sinf[srows, HALF:]
    return (
        cos_ch.astype(np.float16),
        sin_ch.astype(np.float16),
        cosv.astype(np.float16),
        sinv.astype(np.float16),
    )


def make_in_maps(hidden_states, rotary_pos_emb, q_w, k_w, v_w, o_w):
    hs = np.asarray(hidden_states, np.float32)
    q_w = np.asarray(q_w, np.float32)
    k_w = np.asarray(k_w, np.float32)
    v_w = np.asarray(v_w, np.float32)
    o_w = np.asarray(o_w, np.float32)
    cos_t, sin_t, cosv, sinv = _prep_consts(rotary_pos_emb)
    # dedup the expensive transposes: one xt per batch, one weight set per
    # head-group; the 8 per-core maps reference the shared arrays.
    xts = [hs[b].T.astype(np.float16) for b in range(B)]
    wsets = []
    for hg in range(2):
        rows = slice(hg * CPC, hg * CPC + CPC)
        wsets.append(
            {
                "wqT": q_w[rows].T.astype(np.float16),
                "wkT": k_w[rows].T.astype(np.float16),
                "wvT": v_w[rows].T.astype(np.float16),
                "woT": o_w[:, rows].T.astype(np.float16),
            }
        )
    in_maps = []
    for c in range(NCORES):
        b, hg = c // 2, c % 2
        m = {
            "xt": xts[b],
            "cos_t": cos_t,
            "sin_t": sin_t,
            "cosv": cosv,
            "sinv": sinv,
        }
        m.update(wsets[hg])
        in_maps.append(m)
    return in_maps


# ---------------------------------------------------------------------------
# execution: cached jitted runner (modeled on bass2jax.run_bass_via_pjrt but
# reusable across calls and without donated outputs)

_RUNNER = None


def _get_runner():
    global _RUNNER
    if _RUNNER is not None:
        return _RUNNER

    import jax
    from jax.sharding import Mesh, PartitionSpec
    from jax.experimental.shard_map import shard_map
    from concourse import bass2jax

    nc = build_nc()
    bass2jax.install_neuronx_cc_hook()

    partition_name = (
        nc.partition_id_tensor.name if nc.partition_id_tensor else None
    )
    in_names, out_names, out_avals, zero_outs = [], [], [], []
    for alloc in nc.m.functions[0].allocations:
        if not isinstance(alloc, mybir.MemoryLocationSet):
            continue
        name = alloc.memorylocations[0].name
        if alloc.kind == "ExternalInput":
            if name != partition_name:
                in_names.append(name)
        elif alloc.kind == "ExternalOutput":
            shape = tuple(alloc.tensor_shape)
            dtype = mybir.dt.np(alloc.dtype)
            out_names.append(name)
            out_avals.append(jax.core.ShapedArray(shape, dtype))
            zero_outs.append(np.zeros(shape, dtype))
    n_params = len(in_names)
    all_names = list(in_names) + list(out_names)
    if partition_name is not None:
        all_names.append(partition_name)

    def _body(*args):
        operands = list(args)
        if partition_name is not None:
            operands.append(bass2jax.partition_id_tensor())
        outs = bass2jax._bass_exec_p.bind(
            *operands,
            out_avals=tuple(out_avals),
            in_names=tuple(all_names),
            out_names=tuple(out_names),
            lowering_input_output_aliases=(),
            sim_require_finite=True,
            sim_require_nnan=True,
            nc=nc,
        )
        return tuple(outs)

    devices = jax.devices()[:NCORES]
    mesh = Mesh(np.asarray(devices), ("core",))
    n_all = n_params + len(out_names)
    sharded = jax.jit(
        shard_map(
            _body,
            mesh=mesh,
            in_specs=(PartitionSpec("core"),) * n_all,
            out_specs=(PartitionSpec("core"),) * len(out_names),
            check_rep=False,
        )
    )

    # stage the zero output buffers on device ONCE — they are pure staging
    # space, so re-uploading them every call would waste transfer time
    concat_zeros_dev = [
        jax.device_put(
            np.zeros((NCORES * z.shape[0], *z.shape[1:]), z.dtype)
        )
        for z in zero_outs
    ]

    _RUNNER = {
        "sharded": sharded,
        "in_names": in_names,
        "out_names": out_names,
        "out_avals": out_avals,
        "concat_zeros": concat_zeros_dev,
        "nc": nc,
        "all_names": all_names,
        "partition_name": partition_name,
    }
    return _RUNNER


def _run_cores(in_maps):
    r = _get_runner()
    concat_in = [
        np.concatenate([np.asarray(in_maps[c][n]) for c in range(NCORES)], axis=0)
        for n in r["in_names"]
    ]
    out_arrs = r["sharded"](*concat_in, *r["concat_zeros"])
    res = []
    for c in range(NCORES):
        res.append(
            {
                n: np.asarray(out_arrs[i]).reshape(
                    NCORES, *r["out_avals"][i].shape
                )[c]
                for i, n in enumerate(r["out_names"])
            }
        )
    return res


def kernel(hidden_states, rotary_pos_emb, q_w, k_w, v_w, o_w, o_b):
    in_maps = make_in_maps(hidden_states, rotary_pos_emb, q_w, k_w, v_w, o_w)
    res = _run_cores(in_maps)
    o_b = np.asarray(o_b, np.float32)
    out = np.empty((B, S, E), np.float32)
    for b in range(B):
        out[b] = res[2 * b]["out"].astype(np.float32)
        out[b] += res[2 * b + 1]["out"]
        out[b] += o_b
    return out
